# revision 1
# baseline (speedup 1.0000x reference)
"""Trainium2 Bass kernel for nn_CCL_Loss (contrastive loss with gathered
neighbor bank).

Strategy (8 NeuronCores, data parallel over anchor rows):
  - M = V*B = 1024 anchors; core c owns anchors [128c, 128c+128).
  - All column orderings are rotated by 128c per core so that the
    self/partner diagonal blocks sit at fixed offsets; the single SPMD
    program is identical across cores, per-core data differs.
  - The saved_features bank (100k x 128) lives in device HBM (fp16);
    each core gathers its 15*512 neighbor rows with indirect DMAs.
  - Distances via PE matmuls (fp16 operands, fp32 PSUM accumulate),
    f(d) = 1/(1+d) via ACT sqrt + DVE fast reciprocal, sum over k via
    identity-matmul accumulation in PSUM, masked log-softmax tail.
"""

import sys
import numpy as np

sys.path.insert(0, '/opt/trn_rl_repo')

import concourse.bass as bass  # noqa: E402
import concourse.bacc as bacc  # noqa: E402
import concourse.mybir as mybir  # noqa: E402
import concourse.tile as tile  # noqa: E402
from concourse.bass_utils import run_bass_kernel_spmd  # noqa: E402
from concourse.dve_ops import (  # noqa: E402
    RECIPROCAL_APPROX_FAST,
    RECIP_APPROX_FAST_CONSTS,
)

P = 128
B, V, D = 512, 2, 128
M = V * B            # 1024
K = 15               # TOP_K
N_BANK = 100000
NCORES = 8
TEMP = 0.07
ALPHA = 1.0 / (K * TEMP)   # acc = (S + K) * ALPHA
BETA = 1.0 / TEMP          # adc = (r0 + 1) * BETA

F16 = mybir.dt.float16
F32 = mybir.dt.float32
I32 = mybir.dt.int32
AF = mybir.ActivationFunctionType
ALU = mybir.AluOpType

_CACHED_NC = None


def _build():
    nc = bacc.Bacc("TRN2", target_bir_lowering=False, debug=False)
    bank = nc.dram_tensor("bank", [N_BANK, D], F16, kind="ExternalInput")
    gidx = nc.dram_tensor("gidx", [P, 4 * K], I32, kind="ExternalInput")
    n2atr = nc.dram_tensor("n2atr", [P, M], F16, kind="ExternalInput")
    atr = nc.dram_tensor("atr", [P, M], F16, kind="ExternalInput")
    na_row = nc.dram_tensor("na_row", [1, M], F16, kind="ExternalInput")
    na_bias = nc.dram_tensor("na_bias", [P, 1], F32, kind="ExternalInput")
    ident_in = nc.dram_tensor("ident_in", [P, P], F16, kind="ExternalInput")
    ones_in = nc.dram_tensor("ones_in", [P, P], F16, kind="ExternalInput")
    loss_out = nc.dram_tensor("loss", [P, 1], F32, kind="ExternalOutput")

    c_rec = RECIP_APPROX_FAST_CONSTS

    with tile.TileContext(nc) as tc:
        with (
            tc.tile_pool(name="const", bufs=1) as cp,
            tc.tile_pool(name="gp", bufs=1) as gp,
            tc.tile_pool(name="nt", bufs=3) as ntp,
            tc.tile_pool(name="df", bufs=3) as dfp,
            tc.tile_pool(name="rr", bufs=3) as rrp,
            tc.tile_pool(name="tail", bufs=1) as tlp,
            tc.tile_pool(name="tp_ps", bufs=1, space="PSUM") as tp_ps,
            tc.tile_pool(name="row_ps", bufs=2, space="PSUM") as row_ps,
            tc.tile_pool(name="col_ps", bufs=1, space="PSUM") as col_ps,
            tc.tile_pool(name="s_ps", bufs=1, space="PSUM") as s_ps,
        ):
            # ---- constants / inputs ------------------------------------
            n2at = cp.tile([P, M], F16)
            nc.sync.dma_start(n2at[:], n2atr[:, :])
            at = cp.tile([P, M], F16)
            nc.sync.dma_start(at[:], atr[:, :])
            nar = cp.tile([1, M], F16)
            nc.sync.dma_start(nar[:], na_row[:, :])
            nab = cp.tile([P, 1], F32)
            nc.sync.dma_start(nab[:], na_bias[:, :])
            idb = cp.tile([P, P], F16)
            nc.sync.dma_start(idb[:], ident_in[:, :])
            ones = cp.tile([P, P], F16)
            nc.sync.dma_start(ones[:], ones_in[:, :])

            # ---- neighbor gather: 5 tiles x 12 gathers of 128 rows -----
            idx_sb = cp.tile([P, 4 * K], I32)
            nc.sync.dma_start(idx_sb[:], gidx[:, :])
            gts = []
            for j in range(5):
                gt = gp.tile([P, 12, D], F16, tag=f"g{j}")
                gts.append(gt)
                for t in range(12):
                    col = 12 * j + t
                    nc.gpsimd.indirect_dma_start(
                        out=gt[:, t, :], out_offset=None, in_=bank[:, :],
                        in_offset=bass.IndirectOffsetOnAxis(
                            ap=idx_sb[:, col:col + 1], axis=0))

            def g_slice(k, s):
                # gather tile for (k, s): flat column 4k+s
                col = 4 * k + s
                return gts[col // 12][:, col % 12, :]

            # ---- persistent PSUM accumulators --------------------------
            s_row = s_ps.tile([P, B], F32, tag="s_row")
            s_col = s_ps.tile([P, M], F32, tag="s_col")

            # ---- d0: anchor-anchor distances (row side only) -----------
            d0p = col_ps.tile([P, M], F32, tag="colp")
            for h in range(2):
                sl = slice(h * B, (h + 1) * B)
                nc.tensor.matmul(d0p[:, sl], n2at[:, 0:P], at[:, sl],
                                 start=True, stop=False)
                nc.tensor.matmul(d0p[:, sl], ones[0:1, :], nar[:, sl],
                                 start=False, stop=True)
            t0 = tlp.tile([P, M], F32)
            nc.scalar.activation(t0[:], d0p[:], AF.Relu, bias=nab[:])
            d0 = tlp.tile([P, M], F32)
            nc.scalar.activation(d0[:], t0[:], AF.Sqrt)
            u0 = t0  # reuse
            nc.vector.tensor_scalar_add(u0[:], d0[:], 1.0)
            r0 = tlp.tile([P, M], F32)
            nc.vector.reciprocal_approx_fast(out=r0[:], in_=u0[:])

            # ---- k loop ------------------------------------------------
            for k in range(K):
                tp = tp_ps.tile([P, B], F16, tag="tp")
                for s in range(4):
                    nc.tensor.transpose(tp[:, s * P:(s + 1) * P],
                                        g_slice(k, s), idb[:])
                neighT = ntp.tile([P, B], F16, tag="neighT")
                nc.vector.tensor_copy(neighT[:], tp[:])
                nsq = ntp.tile([P, B], F16, tag="nsq")
                nc.scalar.activation(nsq[:], neighT[:], AF.Square)
                scr = ntp.tile([P, D], F32, tag="scr")
                nnb = ntp.tile([P, 1], F32, tag="nnb")
                nc.scalar.activation(scr[:], g_slice(k, 0), AF.Square,
                                     accum_out=nnb[:])

                # row side: [anchors(shard), all neighbors]
                rowp = row_ps.tile([P, B], F32, tag="rowp")
                nc.tensor.matmul(rowp[:], n2at[:, 0:P], neighT[:],
                                 start=True, stop=False)
                nc.tensor.matmul(rowp[:], ones[:], nsq[:],
                                 start=False, stop=True)
                d_row = dfp.tile([P, B], F32, tag="d_row")
                nc.scalar.activation(d_row[:], rowp[:], AF.Sqrt, bias=nab[:])
                u_row = dfp.tile([P, B], F32, tag="u_row")
                nc.vector.tensor_scalar_add(u_row[:], d_row[:], 1.0)
                r_row = rrp.tile([P, B], F16, tag="r_row")
                nc.vector._custom_dve(RECIPROCAL_APPROX_FAST, out=r_row[:],
                                      in0=u_row[:], s0=c_rec["s0"],
                                      s1=c_rec["s1"], imm2=c_rec["imm2"])
                nc.tensor.matmul(s_row[:], idb[:], r_row[:],
                                 start=(k == 0), stop=(k == K - 1))

                # col side: [neighbors(shard), all anchors]
                colp = col_ps.tile([P, M], F32, tag="colp")
                for h in range(2):
                    sl = slice(h * B, (h + 1) * B)
                    nc.tensor.matmul(colp[:, sl], neighT[:, 0:P], n2at[:, sl],
                                     start=True, stop=False)
                    nc.tensor.matmul(colp[:, sl], ones[0:1, :], nar[:, sl],
                                     start=False, stop=True)
                d_col = dfp.tile([P, M], F32, tag="d_col")
                nc.scalar.activation(d_col[:], colp[:], AF.Sqrt, bias=nnb[:])
                u_col = dfp.tile([P, M], F32, tag="u_col")
                nc.vector.tensor_scalar_add(u_col[:], d_col[:], 1.0)
                r_col = rrp.tile([P, M], F16, tag="r_col")
                nc.vector._custom_dve(RECIPROCAL_APPROX_FAST, out=r_col[:],
                                      in0=u_col[:], s0=c_rec["s0"],
                                      s1=c_rec["s1"], imm2=c_rec["imm2"])
                for h in range(2):
                    sl = slice(h * B, (h + 1) * B)
                    nc.tensor.matmul(s_col[:, sl], idb[:], r_col[:, sl],
                                     start=(k == 0), stop=(k == K - 1))

            # ---- tail: summed, logits, masked log-softmax --------------
            # K*ALPHA == BETA == 1/0.07 so one bias constant serves all three
            bias_c = tlp.tile([P, 1], F32)
            nc.vector.memset(bias_c[:], float(BETA))
            acc2r = tlp.tile([P, B], F32)
            nc.scalar.activation(acc2r[:], s_row[:], AF.Square,
                                 bias=bias_c[:], scale=float(ALPHA))
            acc2t = tlp.tile([P, M], F32)
            nc.scalar.activation(acc2t[:], s_col[:], AF.Square,
                                 bias=bias_c[:], scale=float(ALPHA))
            adc2 = tlp.tile([P, M], F32)
            nc.scalar.activation(adc2[:], r0[:], AF.Square,
                                 bias=bias_c[:], scale=float(BETA))
            summed = tlp.tile([P, M], F32)
            for h in range(2):
                sl = slice(h * B, (h + 1) * B)
                nc.vector.tensor_add(summed[:, sl], acc2t[:, sl], acc2r[:])
            for h in range(2):
                sl = slice(h * B, (h + 1) * B)
                nc.vector.tensor_add(summed[:, sl], summed[:, sl], adc2[:, sl])
            logits = tlp.tile([P, M], F32)
            nc.scalar.activation(logits[:], summed[:], AF.Sqrt)

            negm = tlp.tile([P, 1], F32)
            nc.vector.tensor_reduce(negm[:], logits[:], axis=mybir.AxisListType.X,
                                    op=ALU.max, negate=True)
            # self/partner values via identity-masked multiply + reduce
            idf32 = tlp.tile([P, P], F32)
            nc.vector.tensor_copy(idf32[:], idb[:])
            scr2 = tlp.tile([P, P], F32)
            sv = tlp.tile([P, 1], F32)
            nc.vector.tensor_mul(scr2[:], logits[:, 0:P], idf32[:])
            nc.vector.tensor_reduce(sv[:], scr2[:], axis=mybir.AxisListType.X,
                                    op=ALU.add)
            scr3 = tlp.tile([P, P], F32)
            pv = tlp.tile([P, 1], F32)
            nc.vector.tensor_mul(scr3[:], logits[:, B:B + P], idf32[:])
            nc.vector.tensor_reduce(pv[:], scr3[:], axis=mybir.AxisListType.X,
                                    op=ALU.add)

            esc = tlp.tile([P, M], F32)
            efull = tlp.tile([P, 1], F32)
            nc.scalar.activation(esc[:], logits[:], AF.Exp, bias=negm[:],
                                 accum_out=efull[:])
            se = tlp.tile([P, 1], F32)
            nc.scalar.activation(se[:], sv[:], AF.Exp, bias=negm[:])
            ee = tlp.tile([P, 1], F32)
            nc.vector.tensor_sub(ee[:], efull[:], se[:])
            loge = tlp.tile([P, 1], F32)
            nc.scalar.activation(loge[:], ee[:], AF.Ln)
            # loss = (logE - negm) - pv  = m + logE - partner
            lv = tlp.tile([P, 1], F32)
            nc.vector.scalar_tensor_tensor(
                out=lv[:], in0=loge[:], scalar=negm[:], in1=pv[:],
                op0=ALU.subtract, op1=ALU.subtract)
            nc.sync.dma_start(loss_out[:, :], lv[:])
    nc.compile()
    return nc


def _get_nc():
    global _CACHED_NC
    if _CACHED_NC is None:
        _CACHED_NC = _build()
    return _CACHED_NC


def _prepare_in_maps(features, indices, saved_features, rks):
    features = np.asarray(features, dtype=np.float32)
    saved_features = np.asarray(saved_features, dtype=np.float32)
    indices = np.asarray(indices).astype(np.int64)
    rks = np.asarray(rks).astype(np.int64)

    contrast = np.swapaxes(features, 0, 1).reshape(M, D)
    anchors16 = contrast.astype(np.float16)
    anchors = anchors16.astype(np.float32)
    na = (anchors ** 2).sum(-1)                     # [M] fp32, norms of rounded anchors

    bank16 = saved_features.astype(np.float16)
    idx2 = rks[indices, :K].astype(np.int32)        # [B, K]

    ident16 = np.eye(P, dtype=np.float16)
    ones16 = np.ones((P, P), dtype=np.float16)

    in_maps = []
    for c in range(NCORES):
        rot = P * c
        perm = (np.arange(M) + rot) % M             # device col j -> orig anchor
        brot = (np.arange(B) + rot) % B             # device b -> orig b
        at_c = np.ascontiguousarray(anchors[perm].T.astype(np.float16))
        n2at_c = np.ascontiguousarray((-2.0 * anchors[perm]).T.astype(np.float16))
        na_row_c = na[perm][None, :].astype(np.float16)
        na_bias_c = na[perm[0:P]][:, None].astype(np.float32)
        # gather columns: col = 4k+s holds idx2[brot[s*128 : (s+1)*128], k]
        gidx_c = np.empty((P, 4 * K), np.int32)
        for k in range(K):
            for s in range(4):
                gidx_c[:, 4 * k + s] = idx2[brot[s * P:(s + 1) * P], k]
        in_maps.append({
            "bank": bank16,
            "gidx": gidx_c,
            "n2atr": n2at_c,
            "atr": at_c,
            "na_row": na_row_c,
            "na_bias": na_bias_c,
            "ident_in": ident16,
            "ones_in": ones16,
        })
    return in_maps


def run(features, indices, saved_features, rks, **run_kwargs):
    """Run the kernel; returns (scalar_loss, BassKernelResults)."""
    in_maps = _prepare_in_maps(features, indices, saved_features, rks)
    nc = _get_nc()
    res = run_bass_kernel_spmd(nc, in_maps, core_ids=list(range(NCORES)),
                               **run_kwargs)
    total = 0.0
    for r in res.results:
        total += float(r["loss"].sum())
    return np.float32(total / M), res


def kernel(features, indices, saved_features, rks):
    out, _ = run(features, indices, saved_features, rks)
    return out


if __name__ == "__main__":
    # quick self-run with random data
    rng = np.random.default_rng(0)
    feats = rng.standard_normal((B, V, D), dtype=np.float32)
    idx = rng.integers(0, N_BANK, size=(B,)).astype(np.int32)
    bank = rng.standard_normal((N_BANK, D), dtype=np.float32)
    rks_a = rng.integers(0, N_BANK, size=(N_BANK, 50)).astype(np.int32)
    print("loss:", kernel(feats, idx, bank, rks_a))

